# revision 4
# baseline (speedup 1.0000x reference)
"""InterpretableMultiHeadAttention TRN2 kernel — batch-sharded over 8 cores.

B=16,S=T=1024,HID=1024,NH=4,DH=256. Each core handles 2 batches end-to-end
(no collectives). All matmuls run as float32r (TF32-like, ~1.6e-4 rel/mm).
Host pre-transposes q/k/v (-> [h,s]) and weights so the PE never transposes.
Dual-orientation scores: [s,t] for softmax+attn output, [t,s] for attn@v.
"""
import numpy as np
import concourse.bacc as bacc
import concourse.mybir as mybir
import concourse.tile as tile
from concourse.bass_utils import run_bass_kernel_spmd

B, S, HID, NH = 16, 1024, 1024, 4
DH = HID // NH            # 256
BPC = B // 8              # batches per core
F32 = mybir.dt.float32
F32R = mybir.dt.float32r
EXP = mybir.ActivationFunctionType.Exp
_CACHE = {}


def _build():
    nc = bacc.Bacc()
    qT = nc.dram_tensor("qT", [BPC, HID, S], F32, kind="ExternalInput")
    kT = nc.dram_tensor("kT", [BPC, HID, S], F32, kind="ExternalInput")
    vT = nc.dram_tensor("vT", [BPC, HID, S], F32, kind="ExternalInput")
    wqT = nc.dram_tensor("wqT", [NH, HID, DH], F32, kind="ExternalInput")
    wkT = nc.dram_tensor("wkT", [NH, HID, DH], F32, kind="ExternalInput")
    wvT = nc.dram_tensor("wvT", [HID, DH], F32, kind="ExternalInput")
    whT = nc.dram_tensor("whT", [DH, HID], F32, kind="ExternalInput")
    outs = nc.dram_tensor("outs", [BPC, S, HID], F32, kind="ExternalOutput")
    attn = nc.dram_tensor("attn", [BPC, S, NH, S], F32, kind="ExternalOutput")

    with tile.TileContext(nc) as tc:
        with tc.tile_pool(name="wpool", bufs=1) as wpool, \
             tc.tile_pool(name="proj", bufs=1) as proj, \
             tc.tile_pool(name="acts", bufs=1) as acts, \
             tc.tile_pool(name="big", bufs=1) as bigp, \
             tc.tile_pool(name="chunks", bufs=2) as chk, \
             tc.tile_pool(name="small", bufs=2) as sml, \
             tc.tile_pool(name="ps", bufs=4, space="PSUM") as ps:

            # ---- small weights resident; wq/wk streamed per use ----
            wv_sb = wpool.tile([128, 8, DH], F32R)
            nc.gpsimd.dma_start(
                out=wv_sb, in_=wvT.rearrange("(ht p) d -> p ht d", p=128))
            wh_sb = wpool.tile([128, 2, HID], F32R)
            nc.gpsimd.dma_start(
                out=wh_sb, in_=whT.rearrange("(dt p) h -> p dt h", p=128))
            ones4_f = wpool.tile([128, 128], F32)
            nc.vector.memset(ones4_f, 4.0)
            ones4 = wpool.tile([128, 128], F32R)
            nc.scalar.copy(ones4, ones4_f)

            for b in range(BPC):
                # ---- projections ----
                q_sT = proj.tile([128, NH, 2, S], F32R, tag="qsT")
                k_sT = proj.tile([128, NH, 2, S], F32R, tag="ksT")
                v_s = proj.tile([128, 8, DH], F32R, tag="vs")

                for (name, src, wdram, dst) in (
                        ("q", qT, wqT, q_sT), ("k", kT, wkT, k_sT)):
                    a_sb = acts.tile([128, 8, S], F32R, tag="actT",
                                     name=f"a_{name}_{b}")
                    nc.gpsimd.dma_start(
                        out=a_sb,
                        in_=src[b].rearrange("(ht p) s -> p ht s", p=128))
                    for n in range(NH):
                        for dt_i in range(2):
                            wst = chk.tile([128, 8, 128], F32R, tag="wst",
                                           name=f"w_{name}_{b}_{n}_{dt_i}")
                            nc.gpsimd.dma_start(
                                out=wst,
                                in_=wdram[n, :, dt_i * 128:(dt_i + 1) * 128]
                                .rearrange("(ht p) d -> p ht d", p=128))
                            for sc in range(2):
                                pp = ps.tile([128, 512], F32, tag="pp")
                                for ht in range(8):
                                    nc.tensor.matmul(
                                        pp,
                                        wst[:, ht, :],
                                        a_sb[:, ht, sc * 512:(sc + 1) * 512],
                                        start=(ht == 0), stop=(ht == 7))
                                nc.scalar.copy(
                                    dst[:, n, dt_i, sc * 512:(sc + 1) * 512], pp)

                # v projection: v_s[t, dv] (lhsT = vT tiles, rhs = wvT)
                a_sb = acts.tile([128, 8, S], F32R, tag="actT", name=f"a_v_{b}")
                nc.gpsimd.dma_start(
                    out=a_sb, in_=vT[b].rearrange("(ht p) s -> p ht s", p=128))
                for tt in range(8):
                    pp = ps.tile([128, DH], F32, tag="pp")
                    for ht in range(8):
                        nc.tensor.matmul(
                            pp, a_sb[:, ht, tt * 128:(tt + 1) * 128],
                            wv_sb[:, ht, :], start=(ht == 0), stop=(ht == 7))
                    nc.scalar.copy(v_s[:, tt, :], pp)

                meanUT = proj.tile([128, 2, S], F32R, tag="meanUT")

                for n in range(NH):
                    # ---- [s,t] path: scores -> exp -> normalize -> attn out
                    for st in range(8):
                        ex = chk.tile([128, S], F32, tag="ex")
                        acc = sml.tile([128, 2], F32, tag="acc")
                        for tch in range(2):
                            pp = ps.tile([128, 512], F32, tag="pp")
                            for dt_i in range(2):
                                nc.tensor.matmul(
                                    pp,
                                    q_sT[:, n, dt_i, st * 128:(st + 1) * 128],
                                    k_sT[:, n, dt_i, tch * 512:(tch + 1) * 512],
                                    start=(dt_i == 0), stop=(dt_i == 1))
                            nc.scalar.activation(
                                ex[:, tch * 512:(tch + 1) * 512], pp, EXP,
                                scale=0.0625,
                                accum_out=acc[:, tch:tch + 1])
                        rsum = sml.tile([128, 1], F32, tag="rsum")
                        nc.vector.tensor_add(rsum, acc[:, 0:1], acc[:, 1:2])
                        rcp = sml.tile([128, 1], F32, tag="rcp")
                        nc.vector.reciprocal(rcp, rsum)
                        nc.vector.tensor_scalar_mul(ex, ex, rcp)
                        nc.sync.dma_start(
                            out=attn[b, st * 128:(st + 1) * 128, n, :], in_=ex)

                    # ---- [t,s] path per s-half: scoresT -> expT -> attn@v
                    for sc in range(2):
                        exT = bigp.tile([128, 8, 512], F32R, tag="exT",
                                        name=f"exT_{b}_{n}_{sc}")
                        for tt in range(8):
                            pp = ps.tile([128, 512], F32, tag="pp")
                            for dt_i in range(2):
                                nc.tensor.matmul(
                                    pp,
                                    k_sT[:, n, dt_i, tt * 128:(tt + 1) * 128],
                                    q_sT[:, n, dt_i, sc * 512:(sc + 1) * 512],
                                    start=(dt_i == 0), stop=(dt_i == 1))
                            nc.scalar.activation(
                                exT[:, tt, :], pp, EXP, scale=0.0625)

                        bcast = sml.tile([128, 512], F32, tag="bcast",
                                         name=f"bc_{b}_{n}_{sc}")
                        pp = ps.tile([128, 512], F32, tag="pp")
                        for tt in range(8):
                            nc.tensor.matmul(
                                pp, ones4, exT[:, tt, :],
                                start=(tt == 0), stop=(tt == 7))
                        nc.vector.reciprocal(bcast, pp)

                        for dv in range(2):
                            pp = ps.tile([128, 512], F32, tag="pp")
                            for tt in range(8):
                                nc.tensor.matmul(
                                    pp, v_s[:, tt, dv * 128:(dv + 1) * 128],
                                    exT[:, tt, :],
                                    start=(tt == 0), stop=(tt == 7))
                            msl = meanUT[:, dv, sc * 512:(sc + 1) * 512]
                            if n == 0:
                                nc.vector.tensor_mul(msl, pp, bcast)
                            else:
                                tmp = sml.tile([128, 512], F32, tag="tmp")
                                nc.vector.tensor_mul(tmp, pp, bcast)
                                nc.vector.tensor_add(msl, msl, tmp)

                # ---- outs = meanT @ WhT ----
                for st in range(8):
                    ob = chk.tile([128, HID], F32, tag="ob")
                    for hc in range(2):
                        pp = ps.tile([128, 512], F32, tag="out")
                        for dv in range(2):
                            nc.tensor.matmul(
                                pp, meanUT[:, dv, st * 128:(st + 1) * 128],
                                wh_sb[:, dv, hc * 512:(hc + 1) * 512],
                                start=(dv == 0), stop=(dv == 1))
                        nc.scalar.copy(ob[:, hc * 512:(hc + 1) * 512], pp)
                    nc.sync.dma_start(
                        out=outs[b, st * 128:(st + 1) * 128, :], in_=ob)
    nc.finalize()
    return nc


def kernel(q, k, v, Wq, Wk, Wv, Wh):
    q = np.asarray(q, dtype=np.float32)
    k = np.asarray(k, dtype=np.float32)
    v = np.asarray(v, dtype=np.float32)
    wqT = np.ascontiguousarray(np.asarray(Wq, np.float32).transpose(0, 2, 1))
    wkT = np.ascontiguousarray(np.asarray(Wk, np.float32).transpose(0, 2, 1))
    wvT = np.ascontiguousarray(np.asarray(Wv, np.float32).T)
    whT = np.ascontiguousarray(np.asarray(Wh, np.float32).T)

    if "nc" not in _CACHE:
        _CACHE["nc"] = _build()
    nc = _CACHE["nc"]

    in_maps = []
    for c in range(8):
        sl = slice(c * BPC, (c + 1) * BPC)
        in_maps.append({
            "qT": np.ascontiguousarray(q[sl].transpose(0, 2, 1)),
            "kT": np.ascontiguousarray(k[sl].transpose(0, 2, 1)),
            "vT": np.ascontiguousarray(v[sl].transpose(0, 2, 1)),
            "wqT": wqT, "wkT": wkT, "wvT": wvT, "whT": whT,
        })
    res = run_bass_kernel_spmd(nc, in_maps, core_ids=list(range(8)))
    outs = np.concatenate([res.results[c]["outs"] for c in range(8)], axis=0)
    attn = np.concatenate([res.results[c]["attn"] for c in range(8)], axis=0)
    return outs, attn
